# revision 45
# baseline (speedup 1.0000x reference)
"""Trainium2 Bass kernel for nn_CGFA (cross-graph feature aggregation).

Pure data parallel over 8 NeuronCores: B=4096 -> 512 pairs/core, processed in
tiles of G=8 pairs (16 graphs). Host pre-work: embeddings pre-transposed to
feature-major bf16; adjacency shipped as column-normalized A^T (block-diagonal,
2 pairs per 128 partitions) in bf16, so the device never computes column sums
and never runs an fp32 matmul. All PSUM tiles are bf16 single-bank except the
affinity scores (kept f32 for the softmax), halving evacuation cost.

Per-tile layout: "stack" b in 0..7 packs 2 graphs per 128 partitions
(partition = parity*64 + node, parity = pair index & 1); stacks 0-3 are the
src side (pairs 2b, 2b+1), stacks 4-7 the dst side. Feature-major tiles are
[128(d), 1024] with column = side*512 + g*64 + n.
"""

import os
import sys

STAGE = int(os.environ.get("CGFA_STAGE", "6"))

sys.path.insert(0, "/opt/trn_rl_repo")

import numpy as np

from concourse import bass, bacc
import concourse.mybir as mybir
from concourse.bass_utils import run_bass_kernel_spmd
from concourse.tile import TileContext

F32 = mybir.dt.float32
BF = mybir.dt.bfloat16
AF = mybir.ActivationFunctionType
ALU = mybir.AluOpType
AX = mybir.AxisListType

B, N, D = 4096, 64, 128
NCORES = 8
BC = B // NCORES  # 512 pairs per core
G = 8  # pairs per tile


def _emit(nc, n_pairs, with_ba):
    NT = n_pairs // G

    # ---- DRAM I/O ----
    dET = nc.dram_tensor("eT_all", [NT, 128, 1024], BF, kind="ExternalInput").ap()
    dAT = nc.dram_tensor("atn_all", [NT, 128, 8, 128], BF, kind="ExternalInput").ap()
    dWa = nc.dram_tensor("Wa", [D, D], BF, kind="ExternalInput").ap()
    dWu = nc.dram_tensor("Wu", [D, D], BF, kind="ExternalInput").ap()
    dAff = nc.dram_tensor("Aff", [D, D], BF, kind="ExternalInput").ap()
    dWct = nc.dram_tensor("Wct", [D, D], BF, kind="ExternalInput").ap()
    dWcb = nc.dram_tensor("Wcb", [D, D], BF, kind="ExternalInput").ap()
    dWp1 = nc.dram_tensor("Wp1", [D, D], BF, kind="ExternalInput").ap()
    dWp2 = nc.dram_tensor("Wp2", [D, D], BF, kind="ExternalInput").ap()
    dbaW = nc.dram_tensor("baW", [D, D], BF, kind="ExternalInput").ap()
    dbu = nc.dram_tensor("bu_col", [D, 1], F32, kind="ExternalInput").ap()
    dbc = nc.dram_tensor("bc_col", [D, 1], F32, kind="ExternalInput").ap()
    dIb = nc.dram_tensor("ident_bf", [128, 128], BF, kind="ExternalInput").ap()
    dgT = nc.dram_tensor("gT_all", [NT, 128, 16], BF, kind="ExternalOutput").ap()

    with TileContext(nc) as tc:
        with (
            tc.tile_pool(name="const", bufs=1) as cpool,
            tc.tile_pool(name="work", bufs=6) as wpool,
            tc.tile_pool(name="psum", bufs=3, space="PSUM") as ppool,
            tc.tile_pool(name="psums", bufs=2, space="PSUM") as spool,
        ):
            Wa = cpool.tile([128, 128], BF, tag="Wa")
            Wu = cpool.tile([128, 128], BF, tag="Wu")
            Aff = cpool.tile([128, 128], BF, tag="Aff")
            Wct = cpool.tile([128, 128], BF, tag="Wct")
            Wcb = cpool.tile([128, 128], BF, tag="Wcb")
            Wp1 = cpool.tile([128, 128], BF, tag="Wp1")
            Wp2 = cpool.tile([128, 128], BF, tag="Wp2")
            baW = cpool.tile([128, 128], BF, tag="baW")
            Ib = cpool.tile([128, 128], BF, tag="Ib")
            ones = cpool.tile([128, 128], BF, tag="ones")
            bu = cpool.tile([128, 1], F32, tag="bu")
            bc = cpool.tile([128, 1], F32, tag="bc")
            onesbd = cpool.tile([128, 8, 2], BF, tag="onesbd")
            sm_tiles = [cpool.tile([128, 8, 128], BF, tag=f"sm{i}",
                                   name=f"sm{i}") for i in range(3)]
            scbd_tiles = [cpool.tile([128, 8, 2], BF, tag=f"scbd{i}",
                                     name=f"scbd{i}") for i in range(3)]
            loads = [
                (Wa, dWa), (Wu, dWu), (Aff, dAff), (Wct, dWct), (Wcb, dWcb),
                (Wp1, dWp1), (Wp2, dWp2), (Ib, dIb), (bu, dbu), (bc, dbc),
            ]
            if with_ba:
                loads.append((baW, dbaW))
            for tile_, src in loads:
                nc.sync.dma_start(out=tile_[:], in_=src)
            nc.gpsimd.memset(ones[:], 1.0)
            nc.gpsimd.memset(onesbd[:], 0.0)
            nc.gpsimd.memset(onesbd[0:64, :, 0], 1.0)
            nc.gpsimd.memset(onesbd[64:128, :, 1], 1.0)
            for st in sm_tiles + scbd_tiles:
                nc.gpsimd.memset(st[:], 0.0)

            def load(t):
                xT = wpool.tile([128, 1024], BF, tag="xT")
                atn = wpool.tile([128, 8, 128], BF, tag="atn")
                nc.sync.dma_start(out=xT[:], in_=dET[t:t + 1])
                nc.sync.dma_start(out=atn[:], in_=dAT[t:t + 1])
                return xT, atn

            def phase_a(t, xT, atn):
                """Message passing for all 16 graphs -> (e_T [128,1024], e_n)."""
                # ax node-major directly: (x @ Wa)^T^T per 128-token block
                ps_axn = ppool.tile([128, 8, 128], F32, tag="big")
                for b_ in range(8):
                    nc.tensor.matmul(ps_axn[:, b_, :],
                                     xT[:, b_ * 128:(b_ + 1) * 128], Wa[:],
                                     start=True, stop=not with_ba)
                    if with_ba:
                        nc.tensor.matmul(ps_axn[:, b_, :], ones[:], baW[:],
                                         start=False, stop=True)
                axn = wpool.tile([128, 8, 128], BF, tag="axn")
                nc.scalar.activation(axn[:], ps_axn[:], AF.Relu)

                # ux feature-major (bias per-partition here)
                ps_ux = ppool.tile([128, 2, 512], F32, tag="big")
                nc.tensor.matmul(ps_ux[:, 0, :], Wu[:], xT[:, 0:512])
                nc.tensor.matmul(ps_ux[:, 1, :], Wu[:], xT[:, 512:1024])
                uxT = wpool.tile([128, 1024], BF, tag="uxT")
                nc.scalar.activation(
                    uxT[:].rearrange("p (h c) -> p h c", h=2), ps_ux[:],
                    AF.Relu, bias=bu[:, 0:1])

                # e = An @ ax (feature-major); ux added in finish_a later
                ps_e = ppool.tile([128, 8, 128], F32, tag="big")
                for b_ in range(8):
                    nc.tensor.matmul(ps_e[:, b_, :], axn[:, b_, :], atn[:, b_, :])
                return ps_e, uxT

            def finish_a(t, ps_e, uxT):
                """e_T = ps_e + ux^T; node-major copy; t = (e1 @ Aff)^T.
                Emitted after B2a so the PE transposes never wait on the
                DVE add."""
                e_T = wpool.tile([128, 1024], BF, tag="eT")
                nc.vector.tensor_tensor(
                    out=e_T[:].rearrange("p (b c) -> p b c", b=8), in0=ps_e[:],
                    in1=uxT[:].rearrange("p (b c) -> p b c", b=8), op=ALU.add)
                ps_en = spool.tile([128, 8, 128], BF, tag="s")
                for b_ in range(8):
                    nc.tensor.transpose(ps_en[:, b_, :],
                                        e_T[:, b_ * 128:(b_ + 1) * 128], Ib[:])
                e_n = wpool.tile([128, 8, 128], BF, tag="en")
                nc.vector.tensor_copy(e_n[:], ps_en[:])
                ps_t = spool.tile([128, 512], F32, tag="s")
                nc.tensor.matmul(ps_t[:], Aff[:], e_T[:, 0:512])
                tT = wpool.tile([128, 512], BF, tag="tT")
                nc.scalar.copy(tT[:], ps_t[:])
                return e_T, e_n, tT

            def dump_cols(src_T, t):
                """Debug: write col n=0 of each pair (16 cols) to dgT[t]."""
                gT = wpool.tile([128, 16], F32, tag="gT")
                nc.vector.tensor_copy(
                    gT[:], src_T[:].rearrange("p (c n) -> p c n", n=64)[:, :, 0])
                nc.sync.dma_start(out=dgT[t:t + 1], in_=gT[:])

            def pair_b1(t, e_T, e_n, tT):
                """Affinity scores + softmax (both directions, batched)."""
                sm = sm_tiles[t % 3]
                ps_s = spool.tile([128, 8, 64], F32, tag="s")
                for p in range(G):
                    gg, par = p // 2, p % 2
                    sl = slice(par * 64, (par + 1) * 64)
                    tb = tT[:, p * 64:(p + 1) * 64]
                    eb = e_T[:, 512 + p * 64:512 + (p + 1) * 64]
                    nc.tensor.matmul(ps_s[sl, gg, :], tb, eb,
                                     tile_position=(0, par * 64))
                    nc.tensor.matmul(ps_s[sl, 4 + gg, :], eb, tb,
                                     tile_position=(0, par * 64))

                mx = wpool.tile([128, 8], F32, tag="mx")
                nc.vector.reduce_max(mx[:], ps_s[:], axis=AX.X)
                sb = wpool.tile([128, 8, 64], BF, tag="sb")
                nc.vector.tensor_tensor(
                    out=sb[:], in0=ps_s[:],
                    in1=mx[:].to_broadcast([128, 8, 64]), op=ALU.subtract)
                E = wpool.tile([128, 8, 64], BF, tag="E")
                nc.scalar.activation(E[:], sb[:], AF.Exp)
                den = wpool.tile([128, 8], F32, tag="den")
                nc.vector.reduce_sum(den[:], E[:], axis=AX.X)
                rs = wpool.tile([128, 8], F32, tag="rs")
                nc.vector.reciprocal(rs[:], den[:])
                nc.vector.tensor_tensor(
                    out=sm[0:64, :, 0:64], in0=E[0:64, :, :],
                    in1=rs[0:64, :].to_broadcast([64, 8, 64]), op=ALU.mult)
                nc.vector.tensor_tensor(
                    out=sm[64:128, :, 64:128], in0=E[64:128, :, :],
                    in1=rs[64:128, :].to_broadcast([64, 8, 64]), op=ALU.mult)
                return sm

            def pair_b2a(t, e_T, e_n, sm, ps_smT):
                """Softmax transpose + cross-graph aggregation z."""
                for b_ in range(8):
                    nc.tensor.transpose(ps_smT[:, b_, :], sm[:, b_, :], Ib[:])
                smT = wpool.tile([128, 8, 128], BF, tag="smT")
                nc.vector.tensor_copy(smT[:], ps_smT[:, 0:8, :])

                ps_z = ppool.tile([128, 8, 128], F32, tag="big")
                for gg in range(4):
                    nc.tensor.matmul(ps_z[:, gg, :], e_n[:, 4 + gg, :],
                                     smT[:, gg, :])
                    nc.tensor.matmul(ps_z[:, 4 + gg, :], e_n[:, gg, :],
                                     smT[:, 4 + gg, :])
                zT = wpool.tile([128, 1024], BF, tag="zT")
                nc.scalar.copy(
                    zT[:].rearrange("p (b c) -> p b c", b=8), ps_z[:])
                return zT

            def pair_b2b(t, e_T, e_n, zT):
                """Combine, pooling, output (single stage)."""
                if STAGE == 4:
                    dump_cols(zT, t)
                    return
                ps_n = ppool.tile([128, 2, 512], F32, tag="big")
                for h in range(2):
                    nc.tensor.matmul(ps_n[:, h, :], Wct[:],
                                     e_T[:, h * 512:(h + 1) * 512],
                                     start=True, stop=False)
                    nc.tensor.matmul(ps_n[:, h, :], Wcb[:],
                                     zT[:, h * 512:(h + 1) * 512],
                                     start=False, stop=True)
                nT = wpool.tile([128, 1024], BF, tag="nT")
                nc.scalar.activation(
                    nT[:].rearrange("p (h c) -> p h c", h=2), ps_n[:],
                    AF.Identity, bias=bc[:, 0:1])
                if STAGE == 5:
                    dump_cols(nT, t)
                    return
                ps_nn = spool.tile([128, 8, 128], BF, tag="s")
                for b_ in range(8):
                    nc.tensor.transpose(ps_nn[:, b_, :],
                                        nT[:, b_ * 128:(b_ + 1) * 128], Ib[:])
                n_n = wpool.tile([128, 8, 128], BF, tag="nn")
                nc.vector.tensor_copy(n_n[:], ps_nn[:])
                return nT, n_n

            def pool_head(t, nT, n_n):
                ps_pool = spool.tile([128, 512], F32, tag="s")
                for b_ in range(8):
                    nc.tensor.matmul(ps_pool[:, 40 + 2 * b_:42 + 2 * b_],
                                     n_n[:, b_, :], onesbd[:, b_, :])
                msum = wpool.tile([128, 16], BF, tag="msum")
                nc.scalar.copy(msum[:], ps_pool[:, 40:56])
                nc.tensor.matmul(ps_pool[:, 0:8], Wp1[:], msum[:, 0:8])
                nc.tensor.matmul(ps_pool[:, 8:16], Wp2[:], msum[:, 8:16])
                ctxT = wpool.tile([128, 16], BF, tag="ctxT")
                nc.scalar.activation(ctxT[:], ps_pool[:, 0:16], AF.Tanh,
                                     scale=1.0 / N)
                return ps_pool, ctxT

            def pool_tail1(t, nT, ps_pool, ctxT):
                """Attention scores -> sigmoid -> block-diag score columns."""
                for c in range(16):
                    par = c % 2
                    nc.tensor.matmul(
                        ps_pool[par * 64:(par + 1) * 64, 16 + c // 2:17 + c // 2],
                        nT[:, c * 64:(c + 1) * 64], ctxT[:, c:c + 1],
                        tile_position=(0, par * 64))
                esc = wpool.tile([128, 8], F32, tag="esc")
                nc.scalar.activation(esc[:], ps_pool[:, 16:24], AF.Exp,
                                     scale=-1.0)
                nc.gpsimd.tensor_scalar_add(esc[:], esc[:], 1.0)
                rsc = wpool.tile([128, 8], F32, tag="rsc")
                nc.vector.reciprocal(rsc[:], esc[:])
                scbd = scbd_tiles[t % 3]
                nc.gpsimd.tensor_copy(scbd[0:64, :, 0], rsc[0:64, :])
                nc.gpsimd.tensor_copy(scbd[64:128, :, 1], rsc[64:128, :])
                return scbd

            def pool_tail2(t, n_n, scbd, ps_pool):
                """Weighted sums one tile later: scbd is a full tile old."""
                for b_ in range(8):
                    nc.tensor.matmul(ps_pool[:, 24 + 2 * b_:26 + 2 * b_],
                                     n_n[:, b_, :], scbd[:, b_, :])
                gT = wpool.tile([128, 16], BF, tag="gTo")
                nc.scalar.copy(gT[:], ps_pool[:, 24:40])
                nc.sync.dma_start(out=dgT[t:t + 1], in_=gT[:])

            # software pipeline: phase A split around B2a; softmax one tile
            # ahead of its consumer; pooling spread over the iteration tail.
            tiles = {}
            tiles[0] = load(0)
            if NT > 1:
                tiles[1] = load(1)
            cur = finish_a(0, *phase_a(0, *tiles.pop(0)))
            cur_sm = None if STAGE == 2 else pair_b1(0, *cur)
            pool_in = {}
            heads = {}
            sc_out = {}
            for t in range(NT + 2):
                if t + 2 < NT:
                    tiles[t + 2] = load(t + 2)
                if t < NT and STAGE != 2:
                    ps_smT = ppool.tile([128, 16, 128], BF, tag="big",
                                        name="ps_smT")
                pa_nxt = (phase_a(t + 1, *tiles.pop(t + 1))
                          if t + 1 < NT else None)
                if t < NT:
                    if STAGE == 2:
                        dump_cols(cur[0], t)
                    else:
                        zT = pair_b2a(t, cur[0], cur[1], cur_sm, ps_smT)
                nxt = finish_a(t + 1, *pa_nxt) if pa_nxt is not None else None
                if 0 <= t - 1 < NT and (t - 1) in pool_in:
                    heads[t - 1] = pool_head(t - 1, *pool_in[t - 1])
                nxt_sm = (pair_b1(t + 1, *nxt)
                          if nxt is not None and STAGE != 2 else None)
                if t < NT and STAGE != 2:
                    r = pair_b2b(t, cur[0], cur[1], zT)
                    if r is not None:
                        pool_in[t] = r
                cur, cur_sm = nxt, nxt_sm
                cur_pool = None
                u = t - 1
                if 0 <= u < NT and u in pool_in:
                    cur_pool = spool.tile([128, 512], F32, tag="s",
                                          name="cur_pool")
                    sc_out[u] = pool_tail1(u, pool_in[u][0], cur_pool,
                                           heads.pop(u)[1])
                u = t - 2
                if 0 <= u < NT and u in sc_out:
                    if cur_pool is None:
                        cur_pool = spool.tile([128, 512], F32, tag="s",
                                              name="cur_pool")
                    pool_tail2(u, pool_in.pop(u)[1], sc_out.pop(u), cur_pool)
    nc.finalize()
    return nc


_BUILT = {}


def _get_nc(n_pairs, with_ba=False):
    key = (n_pairs, with_ba)
    if key not in _BUILT:
        nc = bacc.Bacc("TRN2", target_bir_lowering=False, debug=False,
                       num_devices=NCORES)
        _BUILT[key] = _emit(nc, n_pairs, with_ba)
    return _BUILT[key]


def _prep_side(ml, eT, atn, side, A, emb):
    """Host: bf16 feature-major emb + column-normalized block-diag A^T."""
    bf = ml.bfloat16
    NTt = eT.shape[0]
    ee = np.asarray(emb, np.float32).reshape(NTt, G, 64, 128)
    eT[:, :, side * 512:(side + 1) * 512] = (
        ee.transpose(0, 3, 1, 2).reshape(NTt, 128, 512).astype(bf))
    A = np.asarray(A, np.float32)
    An = A / np.clip(A.sum(axis=1, keepdims=True), 1e-12, None)
    AT = An.transpose(0, 2, 1).reshape(NTt, 4, 2, 64, 64).astype(bf)
    atn[:, 0:64, side * 4:(side + 1) * 4, 0:64] = AT[:, :, 0].transpose(0, 2, 1, 3)
    atn[:, 64:128, side * 4:(side + 1) * 4, 64:128] = AT[:, :, 1].transpose(0, 2, 1, 3)


def kernel(A_src, emb_src, mask_src, A_dst, emb_dst, mask_dst,
           Wa, ba, Wu, bu, Aff, Wc, bc, Wp1, Wp2):
    import ml_dtypes as ml
    bf = ml.bfloat16

    Bt = np.asarray(A_src).shape[0]
    n_pairs = Bt // NCORES
    NTt = Bt // G
    ba = np.asarray(ba, np.float32)
    with_ba = bool(np.abs(ba).max() > 0)
    nc = _get_nc(n_pairs, with_ba)

    eT = np.empty((NTt, 128, 1024), dtype=bf)
    atn = np.zeros((NTt, 128, 8, 128), dtype=bf)
    _prep_side(ml, eT, atn, 0, A_src, emb_src)
    _prep_side(ml, eT, atn, 1, A_dst, emb_dst)

    shared = {
        "Wa": np.asarray(Wa, bf),
        "Wu": np.asarray(Wu, bf),
        "Aff": np.asarray(Aff, bf),
        "Wct": np.ascontiguousarray(np.asarray(Wc, np.float32)[:D]).astype(bf),
        "Wcb": np.ascontiguousarray(np.asarray(Wc, np.float32)[D:]).astype(bf),
        "Wp1": np.asarray(Wp1, bf),
        "Wp2": np.asarray(Wp2, bf),
        "baW": np.tile((ba / 128.0)[None, :], (128, 1)).astype(bf),
        "bu_col": np.ascontiguousarray(np.asarray(bu, np.float32)[:, None]),
        "bc_col": np.ascontiguousarray(np.asarray(bc, np.float32)[:, None]),
        "ident_bf": np.eye(128, dtype=bf),
    }
    NTc = n_pairs // G
    in_maps = []
    for c in range(NCORES):
        sl = slice(c * NTc, (c + 1) * NTc)
        in_maps.append({
            "eT_all": np.ascontiguousarray(eT[sl]),
            "atn_all": np.ascontiguousarray(atn[sl]),
            **shared,
        })
    res = run_bass_kernel_spmd(nc, in_maps, list(range(NCORES)))
    gs = [np.asarray(res.results[c]["gT_all"]).astype(np.float32)
          for c in range(NCORES)]
    gT_all = np.concatenate(gs, axis=0)  # [NTt, 128, 16]
    g1 = gT_all[:, :, 0:8].transpose(0, 2, 1).reshape(Bt, 128)
    g2 = gT_all[:, :, 8:16].transpose(0, 2, 1).reshape(Bt, 128)
    return (np.ascontiguousarray(g1), np.ascontiguousarray(g2))


# revision 47
# speedup vs baseline: 1.0001x; 1.0001x over previous
"""Trainium2 Bass kernel for nn_CGFA (cross-graph feature aggregation).

Pure data parallel over 8 NeuronCores: B=4096 -> 512 pairs/core, processed in
tiles of G=8 pairs (16 graphs). Host pre-work: embeddings pre-transposed to
feature-major bf16; adjacency shipped as column-normalized A^T (block-diagonal,
2 pairs per 128 partitions) in bf16, so the device never computes column sums
and never runs an fp32 matmul. All PSUM tiles are bf16 single-bank except the
affinity scores (kept f32 for the softmax), halving evacuation cost.

Per-tile layout: "stack" b in 0..7 packs 2 graphs per 128 partitions
(partition = parity*64 + node, parity = pair index & 1); stacks 0-3 are the
src side (pairs 2b, 2b+1), stacks 4-7 the dst side. Feature-major tiles are
[128(d), 1024] with column = side*512 + g*64 + n.
"""

import os
import sys

STAGE = int(os.environ.get("CGFA_STAGE", "6"))

sys.path.insert(0, "/opt/trn_rl_repo")

import numpy as np

from concourse import bass, bacc
import concourse.mybir as mybir
from concourse.bass_utils import run_bass_kernel_spmd
from concourse.tile import TileContext

F32 = mybir.dt.float32
BF = mybir.dt.bfloat16
AF = mybir.ActivationFunctionType
ALU = mybir.AluOpType
AX = mybir.AxisListType

B, N, D = 4096, 64, 128
NCORES = 8
BC = B // NCORES  # 512 pairs per core
G = 8  # pairs per tile


def _emit(nc, n_pairs, with_ba):
    NT = n_pairs // G

    # ---- DRAM I/O ----
    dET = nc.dram_tensor("eT_all", [NT, 128, 1024], BF, kind="ExternalInput").ap()
    dAT = nc.dram_tensor("atn_all", [NT, 128, 8, 128], BF, kind="ExternalInput").ap()
    dWa = nc.dram_tensor("Wa", [D, D], BF, kind="ExternalInput").ap()
    dWu = nc.dram_tensor("Wu", [D, D], BF, kind="ExternalInput").ap()
    dAff = nc.dram_tensor("Aff", [D, D], BF, kind="ExternalInput").ap()
    dWct = nc.dram_tensor("Wct", [D, D], BF, kind="ExternalInput").ap()
    dWcb = nc.dram_tensor("Wcb", [D, D], BF, kind="ExternalInput").ap()
    dWp1 = nc.dram_tensor("Wp1", [D, D], BF, kind="ExternalInput").ap()
    dWp2 = nc.dram_tensor("Wp2", [D, D], BF, kind="ExternalInput").ap()
    dbaW = nc.dram_tensor("baW", [D, D], BF, kind="ExternalInput").ap()
    dbu = nc.dram_tensor("bu_col", [D, 1], F32, kind="ExternalInput").ap()
    dbc = nc.dram_tensor("bc_col", [D, 1], F32, kind="ExternalInput").ap()
    dIb = nc.dram_tensor("ident_bf", [128, 128], BF, kind="ExternalInput").ap()
    dgT = nc.dram_tensor("gT_all", [NT, 128, 16], BF, kind="ExternalOutput").ap()

    with TileContext(nc) as tc:
        with (
            tc.tile_pool(name="const", bufs=1) as cpool,
            tc.tile_pool(name="work", bufs=5) as wpool,
            tc.tile_pool(name="psum", bufs=3, space="PSUM") as ppool,
            tc.tile_pool(name="psums", bufs=2, space="PSUM") as spool,
        ):
            Wa = cpool.tile([128, 128], BF, tag="Wa")
            Wu = cpool.tile([128, 128], BF, tag="Wu")
            Aff = cpool.tile([128, 128], BF, tag="Aff")
            Wct = cpool.tile([128, 128], BF, tag="Wct")
            Wcb = cpool.tile([128, 128], BF, tag="Wcb")
            Wp1 = cpool.tile([128, 128], BF, tag="Wp1")
            Wp2 = cpool.tile([128, 128], BF, tag="Wp2")
            baW = cpool.tile([128, 128], BF, tag="baW")
            Ib = cpool.tile([128, 128], BF, tag="Ib")
            ones = cpool.tile([128, 128], BF, tag="ones")
            bu = cpool.tile([128, 1], F32, tag="bu")
            bc = cpool.tile([128, 1], F32, tag="bc")
            onesbd = cpool.tile([128, 8, 2], BF, tag="onesbd")
            sm_tiles = [cpool.tile([128, 8, 128], BF, tag=f"sm{i}",
                                   name=f"sm{i}") for i in range(3)]
            scbd_tiles = [cpool.tile([128, 8, 2], BF, tag=f"scbd{i}",
                                     name=f"scbd{i}") for i in range(3)]
            loads = [
                (Wa, dWa), (Wu, dWu), (Aff, dAff), (Wct, dWct), (Wcb, dWcb),
                (Wp1, dWp1), (Wp2, dWp2), (Ib, dIb), (bu, dbu), (bc, dbc),
            ]
            if with_ba:
                loads.append((baW, dbaW))
            for tile_, src in loads:
                nc.sync.dma_start(out=tile_[:], in_=src)
            nc.gpsimd.memset(ones[:], 1.0)
            nc.gpsimd.memset(onesbd[:], 0.0)
            nc.gpsimd.memset(onesbd[0:64, :, 0], 1.0)
            nc.gpsimd.memset(onesbd[64:128, :, 1], 1.0)
            for st in sm_tiles + scbd_tiles:
                nc.gpsimd.memset(st[:], 0.0)

            def load(t):
                xT = wpool.tile([128, 1024], BF, tag="xT")
                atn = wpool.tile([128, 8, 128], BF, tag="atn")
                nc.sync.dma_start(out=xT[:], in_=dET[t:t + 1])
                nc.sync.dma_start(out=atn[:], in_=dAT[t:t + 1])
                return xT, atn

            def phase_a(t, xT, atn):
                """Message passing for all 16 graphs -> (e_T [128,1024], e_n)."""
                # ax node-major directly: (x @ Wa)^T^T per 128-token block
                ps_axn = ppool.tile([128, 8, 128], F32, tag="big")
                for b_ in range(8):
                    nc.tensor.matmul(ps_axn[:, b_, :],
                                     xT[:, b_ * 128:(b_ + 1) * 128], Wa[:],
                                     start=True, stop=not with_ba)
                    if with_ba:
                        nc.tensor.matmul(ps_axn[:, b_, :], ones[:], baW[:],
                                         start=False, stop=True)
                axn = wpool.tile([128, 8, 128], BF, tag="axn")
                nc.scalar.activation(axn[:], ps_axn[:], AF.Relu)

                # ux feature-major (bias per-partition here)
                ps_ux = ppool.tile([128, 2, 512], F32, tag="big")
                nc.tensor.matmul(ps_ux[:, 0, :], Wu[:], xT[:, 0:512])
                nc.tensor.matmul(ps_ux[:, 1, :], Wu[:], xT[:, 512:1024])
                uxT = wpool.tile([128, 1024], BF, tag="uxT")
                nc.scalar.activation(
                    uxT[:].rearrange("p (h c) -> p h c", h=2), ps_ux[:],
                    AF.Relu, bias=bu[:, 0:1])

                # e = An @ ax (feature-major); ux added in finish_a later
                ps_e = ppool.tile([128, 8, 128], F32, tag="big")
                for b_ in range(8):
                    nc.tensor.matmul(ps_e[:, b_, :], axn[:, b_, :], atn[:, b_, :])
                return ps_e, uxT

            def finish_a(t, ps_e, uxT):
                """e_T = ps_e + ux^T; node-major copy; t = (e1 @ Aff)^T.
                Emitted after B2a so the PE transposes never wait on the
                DVE add."""
                e_T = wpool.tile([128, 1024], BF, tag="eT")
                nc.vector.tensor_tensor(
                    out=e_T[:].rearrange("p (b c) -> p b c", b=8), in0=ps_e[:],
                    in1=uxT[:].rearrange("p (b c) -> p b c", b=8), op=ALU.add)
                ps_en = spool.tile([128, 8, 128], BF, tag="s")
                for b_ in range(8):
                    nc.tensor.transpose(ps_en[:, b_, :],
                                        e_T[:, b_ * 128:(b_ + 1) * 128], Ib[:])
                e_n = wpool.tile([128, 8, 128], BF, tag="en")
                nc.vector.tensor_copy(e_n[:], ps_en[:])
                ps_t = spool.tile([128, 512], F32, tag="s")
                nc.tensor.matmul(ps_t[:], Aff[:], e_T[:, 0:512])
                tT = wpool.tile([128, 512], BF, tag="tT")
                nc.scalar.copy(tT[:], ps_t[:])
                return e_T, e_n, tT

            def dump_cols(src_T, t):
                """Debug: write col n=0 of each pair (16 cols) to dgT[t]."""
                gT = wpool.tile([128, 16], F32, tag="gT")
                nc.vector.tensor_copy(
                    gT[:], src_T[:].rearrange("p (c n) -> p c n", n=64)[:, :, 0])
                nc.sync.dma_start(out=dgT[t:t + 1], in_=gT[:])

            def pair_b1(t, e_T, e_n, tT):
                """Affinity scores + softmax (both directions, batched)."""
                sm = sm_tiles[t % 3]
                ps_s = spool.tile([128, 8, 64], F32, tag="s")
                for p in range(G):
                    gg, par = p // 2, p % 2
                    sl = slice(par * 64, (par + 1) * 64)
                    tb = tT[:, p * 64:(p + 1) * 64]
                    eb = e_T[:, 512 + p * 64:512 + (p + 1) * 64]
                    nc.tensor.matmul(ps_s[sl, gg, :], tb, eb,
                                     tile_position=(0, par * 64))
                    nc.tensor.matmul(ps_s[sl, 4 + gg, :], eb, tb,
                                     tile_position=(0, par * 64))

                mx = wpool.tile([128, 8], F32, tag="mx")
                nc.vector.reduce_max(mx[:], ps_s[:], axis=AX.X)
                sb = wpool.tile([128, 8, 64], BF, tag="sb")
                nc.vector.tensor_tensor(
                    out=sb[:], in0=ps_s[:],
                    in1=mx[:].to_broadcast([128, 8, 64]), op=ALU.subtract)
                E = wpool.tile([128, 8, 64], BF, tag="E")
                nc.scalar.activation(E[:], sb[:], AF.Exp)
                den = wpool.tile([128, 8], F32, tag="den")
                nc.vector.reduce_sum(den[:], E[:], axis=AX.X)
                rs = wpool.tile([128, 8], F32, tag="rs")
                nc.vector.reciprocal(rs[:], den[:])
                nc.vector.tensor_tensor(
                    out=sm[0:64, :, 0:64], in0=E[0:64, :, :],
                    in1=rs[0:64, :].to_broadcast([64, 8, 64]), op=ALU.mult)
                nc.vector.tensor_tensor(
                    out=sm[64:128, :, 64:128], in0=E[64:128, :, :],
                    in1=rs[64:128, :].to_broadcast([64, 8, 64]), op=ALU.mult)
                return sm

            def pair_b2a(t, e_T, e_n, sm, ps_smT):
                """Softmax transpose + cross-graph aggregation z."""
                for b_ in range(8):
                    nc.tensor.transpose(ps_smT[:, b_, :], sm[:, b_, :], Ib[:])
                smT = wpool.tile([128, 8, 128], BF, tag="smT")
                nc.vector.tensor_copy(smT[:], ps_smT[:, 0:8, :])

                ps_z = ppool.tile([128, 8, 128], F32, tag="big")
                for gg in range(4):
                    nc.tensor.matmul(ps_z[:, gg, :], e_n[:, 4 + gg, :],
                                     smT[:, gg, :])
                    nc.tensor.matmul(ps_z[:, 4 + gg, :], e_n[:, gg, :],
                                     smT[:, 4 + gg, :])
                zT = wpool.tile([128, 1024], BF, tag="zT")
                nc.scalar.copy(
                    zT[:].rearrange("p (b c) -> p b c", b=8), ps_z[:])
                return zT

            def pair_b2b(t, e_T, e_n, zT):
                """Combine, pooling, output (single stage)."""
                if STAGE == 4:
                    dump_cols(zT, t)
                    return
                ps_n = ppool.tile([128, 2, 512], F32, tag="big")
                for h in range(2):
                    nc.tensor.matmul(ps_n[:, h, :], Wct[:],
                                     e_T[:, h * 512:(h + 1) * 512],
                                     start=True, stop=False)
                    nc.tensor.matmul(ps_n[:, h, :], Wcb[:],
                                     zT[:, h * 512:(h + 1) * 512],
                                     start=False, stop=True)
                nT = wpool.tile([128, 1024], BF, tag="nT")
                nc.scalar.activation(
                    nT[:].rearrange("p (h c) -> p h c", h=2), ps_n[:],
                    AF.Identity, bias=bc[:, 0:1])
                if STAGE == 5:
                    dump_cols(nT, t)
                    return
                ps_nn = spool.tile([128, 8, 128], BF, tag="s")
                for b_ in range(8):
                    nc.tensor.transpose(ps_nn[:, b_, :],
                                        nT[:, b_ * 128:(b_ + 1) * 128], Ib[:])
                n_n = wpool.tile([128, 8, 128], BF, tag="nn")
                nc.vector.tensor_copy(n_n[:], ps_nn[:])
                return nT, n_n

            def pool_head(t, nT, n_n):
                ps_pool = spool.tile([128, 512], F32, tag="s")
                for b_ in range(8):
                    nc.tensor.matmul(ps_pool[:, 40 + 2 * b_:42 + 2 * b_],
                                     n_n[:, b_, :], onesbd[:, b_, :])
                msum = wpool.tile([128, 16], BF, tag="msum")
                nc.scalar.copy(msum[:], ps_pool[:, 40:56])
                nc.tensor.matmul(ps_pool[:, 0:8], Wp1[:], msum[:, 0:8])
                nc.tensor.matmul(ps_pool[:, 8:16], Wp2[:], msum[:, 8:16])
                ctxT = wpool.tile([128, 16], BF, tag="ctxT")
                nc.scalar.activation(ctxT[:], ps_pool[:, 0:16], AF.Tanh,
                                     scale=1.0 / N)
                return ps_pool, ctxT

            def pool_tail1(t, nT, ps_pool, ctxT):
                """Attention scores -> sigmoid -> block-diag score columns."""
                for c in range(16):
                    par = c % 2
                    nc.tensor.matmul(
                        ps_pool[par * 64:(par + 1) * 64, 16 + c // 2:17 + c // 2],
                        nT[:, c * 64:(c + 1) * 64], ctxT[:, c:c + 1],
                        tile_position=(0, par * 64))
                esc = wpool.tile([128, 8], F32, tag="esc")
                nc.scalar.activation(esc[:], ps_pool[:, 16:24], AF.Exp,
                                     scale=-1.0)
                nc.gpsimd.tensor_scalar_add(esc[:], esc[:], 1.0)
                rsc = wpool.tile([128, 8], F32, tag="rsc")
                nc.vector.reciprocal(rsc[:], esc[:])
                scbd = scbd_tiles[t % 3]
                nc.gpsimd.tensor_copy(scbd[0:64, :, 0], rsc[0:64, :])
                nc.gpsimd.tensor_copy(scbd[64:128, :, 1], rsc[64:128, :])
                return scbd

            def pool_tail2(t, n_n, scbd, ps_pool):
                """Weighted sums one tile later: scbd is a full tile old."""
                for b_ in range(8):
                    nc.tensor.matmul(ps_pool[:, 24 + 2 * b_:26 + 2 * b_],
                                     n_n[:, b_, :], scbd[:, b_, :])
                gT = wpool.tile([128, 16], BF, tag="gTo")
                nc.scalar.copy(gT[:], ps_pool[:, 24:40])
                nc.sync.dma_start(out=dgT[t:t + 1], in_=gT[:])

            # software pipeline: phase A split around B2a; softmax one tile
            # ahead of its consumer; pooling spread over the iteration tail.
            tiles = {}
            for i in range(min(3, NT)):
                tiles[i] = load(i)
            cur = finish_a(0, *phase_a(0, *tiles.pop(0)))
            cur_sm = None if STAGE == 2 else pair_b1(0, *cur)
            pool_in = {}
            heads = {}
            sc_out = {}
            for t in range(NT + 2):
                if t + 3 < NT:
                    tiles[t + 3] = load(t + 3)
                if t < NT and STAGE != 2:
                    ps_smT = ppool.tile([128, 16, 128], BF, tag="big",
                                        name="ps_smT")
                pa_nxt = (phase_a(t + 1, *tiles.pop(t + 1))
                          if t + 1 < NT else None)
                if t < NT:
                    if STAGE == 2:
                        dump_cols(cur[0], t)
                    else:
                        zT = pair_b2a(t, cur[0], cur[1], cur_sm, ps_smT)
                nxt = finish_a(t + 1, *pa_nxt) if pa_nxt is not None else None
                if 0 <= t - 1 < NT and (t - 1) in pool_in:
                    heads[t - 1] = pool_head(t - 1, *pool_in[t - 1])
                nxt_sm = (pair_b1(t + 1, *nxt)
                          if nxt is not None and STAGE != 2 else None)
                if t < NT and STAGE != 2:
                    r = pair_b2b(t, cur[0], cur[1], zT)
                    if r is not None:
                        pool_in[t] = r
                cur, cur_sm = nxt, nxt_sm
                cur_pool = None
                u = t - 1
                if 0 <= u < NT and u in pool_in:
                    cur_pool = spool.tile([128, 512], F32, tag="s",
                                          name="cur_pool")
                    sc_out[u] = pool_tail1(u, pool_in[u][0], cur_pool,
                                           heads.pop(u)[1])
                u = t - 2
                if 0 <= u < NT and u in sc_out:
                    if cur_pool is None:
                        cur_pool = spool.tile([128, 512], F32, tag="s",
                                              name="cur_pool")
                    pool_tail2(u, pool_in.pop(u)[1], sc_out.pop(u), cur_pool)
    nc.finalize()
    return nc


_BUILT = {}


def _get_nc(n_pairs, with_ba=False):
    key = (n_pairs, with_ba)
    if key not in _BUILT:
        nc = bacc.Bacc("TRN2", target_bir_lowering=False, debug=False,
                       num_devices=NCORES)
        _BUILT[key] = _emit(nc, n_pairs, with_ba)
    return _BUILT[key]


def _prep_side(ml, eT, atn, side, A, emb):
    """Host: bf16 feature-major emb + column-normalized block-diag A^T."""
    bf = ml.bfloat16
    NTt = eT.shape[0]
    ee = np.asarray(emb, np.float32).reshape(NTt, G, 64, 128)
    eT[:, :, side * 512:(side + 1) * 512] = (
        ee.transpose(0, 3, 1, 2).reshape(NTt, 128, 512).astype(bf))
    A = np.asarray(A, np.float32)
    An = A / np.clip(A.sum(axis=1, keepdims=True), 1e-12, None)
    AT = An.transpose(0, 2, 1).reshape(NTt, 4, 2, 64, 64).astype(bf)
    atn[:, 0:64, side * 4:(side + 1) * 4, 0:64] = AT[:, :, 0].transpose(0, 2, 1, 3)
    atn[:, 64:128, side * 4:(side + 1) * 4, 64:128] = AT[:, :, 1].transpose(0, 2, 1, 3)


def kernel(A_src, emb_src, mask_src, A_dst, emb_dst, mask_dst,
           Wa, ba, Wu, bu, Aff, Wc, bc, Wp1, Wp2):
    import ml_dtypes as ml
    bf = ml.bfloat16

    Bt = np.asarray(A_src).shape[0]
    n_pairs = Bt // NCORES
    NTt = Bt // G
    ba = np.asarray(ba, np.float32)
    with_ba = bool(np.abs(ba).max() > 0)
    nc = _get_nc(n_pairs, with_ba)

    eT = np.empty((NTt, 128, 1024), dtype=bf)
    atn = np.zeros((NTt, 128, 8, 128), dtype=bf)
    _prep_side(ml, eT, atn, 0, A_src, emb_src)
    _prep_side(ml, eT, atn, 1, A_dst, emb_dst)

    shared = {
        "Wa": np.asarray(Wa, bf),
        "Wu": np.asarray(Wu, bf),
        "Aff": np.asarray(Aff, bf),
        "Wct": np.ascontiguousarray(np.asarray(Wc, np.float32)[:D]).astype(bf),
        "Wcb": np.ascontiguousarray(np.asarray(Wc, np.float32)[D:]).astype(bf),
        "Wp1": np.asarray(Wp1, bf),
        "Wp2": np.asarray(Wp2, bf),
        "baW": np.tile((ba / 128.0)[None, :], (128, 1)).astype(bf),
        "bu_col": np.ascontiguousarray(np.asarray(bu, np.float32)[:, None]),
        "bc_col": np.ascontiguousarray(np.asarray(bc, np.float32)[:, None]),
        "ident_bf": np.eye(128, dtype=bf),
    }
    NTc = n_pairs // G
    in_maps = []
    for c in range(NCORES):
        sl = slice(c * NTc, (c + 1) * NTc)
        in_maps.append({
            "eT_all": np.ascontiguousarray(eT[sl]),
            "atn_all": np.ascontiguousarray(atn[sl]),
            **shared,
        })
    res = run_bass_kernel_spmd(nc, in_maps, list(range(NCORES)))
    gs = [np.asarray(res.results[c]["gT_all"]).astype(np.float32)
          for c in range(NCORES)]
    gT_all = np.concatenate(gs, axis=0)  # [NTt, 128, 16]
    g1 = gT_all[:, :, 0:8].transpose(0, 2, 1).reshape(Bt, 128)
    g2 = gT_all[:, :, 8:16].transpose(0, 2, 1).reshape(Bt, 128)
    return (np.ascontiguousarray(g1), np.ascontiguousarray(g2))


# revision 49
# speedup vs baseline: 1.0501x; 1.0500x over previous
"""Trainium2 Bass kernel for nn_CGFA (cross-graph feature aggregation).

Pure data parallel over 8 NeuronCores: B=4096 -> 512 pairs/core, processed in
tiles of G=8 pairs (16 graphs). Host pre-work: embeddings pre-transposed to
feature-major bf16; adjacency shipped as column-normalized A^T (block-diagonal,
2 pairs per 128 partitions) in bf16, so the device never computes column sums
and never runs an fp32 matmul. All PSUM tiles are bf16 single-bank except the
affinity scores (kept f32 for the softmax), halving evacuation cost.

Per-tile layout: "stack" b in 0..7 packs 2 graphs per 128 partitions
(partition = parity*64 + node, parity = pair index & 1); stacks 0-3 are the
src side (pairs 2b, 2b+1), stacks 4-7 the dst side. Feature-major tiles are
[128(d), 1024] with column = side*512 + g*64 + n.
"""

import os
import sys

STAGE = int(os.environ.get("CGFA_STAGE", "6"))

sys.path.insert(0, "/opt/trn_rl_repo")

import numpy as np

from concourse import bass, bacc
import concourse.mybir as mybir
from concourse.bass_utils import run_bass_kernel_spmd
from concourse.tile import TileContext

F32 = mybir.dt.float32
BF = mybir.dt.bfloat16
AF = mybir.ActivationFunctionType
ALU = mybir.AluOpType
AX = mybir.AxisListType

B, N, D = 4096, 64, 128
NCORES = 8
BC = B // NCORES  # 512 pairs per core
G = 8  # pairs per tile


def _emit(nc, n_pairs, with_ba):
    NT = n_pairs // G

    # ---- DRAM I/O ----
    dET = nc.dram_tensor("eT_all", [NT, 128, 1024], BF, kind="ExternalInput").ap()
    dAT = nc.dram_tensor("atn_all", [NT, 128, 8, 128], BF, kind="ExternalInput").ap()
    dWa = nc.dram_tensor("Wa", [D, D], BF, kind="ExternalInput").ap()
    dWu = nc.dram_tensor("Wu", [D, D], BF, kind="ExternalInput").ap()
    dAff = nc.dram_tensor("Aff", [D, D], BF, kind="ExternalInput").ap()
    dWct = nc.dram_tensor("Wct", [D, D], BF, kind="ExternalInput").ap()
    dWcb = nc.dram_tensor("Wcb", [D, D], BF, kind="ExternalInput").ap()
    dWp1 = nc.dram_tensor("Wp1", [D, D], BF, kind="ExternalInput").ap()
    dWp2 = nc.dram_tensor("Wp2", [D, D], BF, kind="ExternalInput").ap()
    dbaW = nc.dram_tensor("baW", [D, D], BF, kind="ExternalInput").ap()
    dbu = nc.dram_tensor("bu_col", [D, 1], F32, kind="ExternalInput").ap()
    dbc = nc.dram_tensor("bc_col", [D, 1], F32, kind="ExternalInput").ap()
    dIb = nc.dram_tensor("ident_bf", [128, 128], BF, kind="ExternalInput").ap()
    dgT = nc.dram_tensor("gT_all", [NT, 128, 16], BF, kind="ExternalOutput").ap()

    with TileContext(nc) as tc:
        with (
            tc.tile_pool(name="const", bufs=1) as cpool,
            tc.tile_pool(name="work", bufs=5) as wpool,
            tc.tile_pool(name="psum", bufs=3, space="PSUM") as ppool,
            tc.tile_pool(name="psums", bufs=2, space="PSUM") as spool,
        ):
            Wa = cpool.tile([128, 128], BF, tag="Wa")
            Wu = cpool.tile([128, 128], BF, tag="Wu")
            Aff = cpool.tile([128, 128], BF, tag="Aff")
            Wct = cpool.tile([128, 128], BF, tag="Wct")
            Wcb = cpool.tile([128, 128], BF, tag="Wcb")
            Wp1 = cpool.tile([128, 128], BF, tag="Wp1")
            Wp2 = cpool.tile([128, 128], BF, tag="Wp2")
            baW = cpool.tile([128, 128], BF, tag="baW")
            Ib = cpool.tile([128, 128], BF, tag="Ib")
            ones = cpool.tile([128, 128], BF, tag="ones")
            bu = cpool.tile([128, 1], F32, tag="bu")
            bc = cpool.tile([128, 1], F32, tag="bc")
            onesbd = cpool.tile([128, 8, 2], BF, tag="onesbd")
            sm_tiles = [cpool.tile([128, 8, 128], BF, tag=f"sm{i}",
                                   name=f"sm{i}") for i in range(3)]
            scbd_tiles = [cpool.tile([128, 8, 2], BF, tag=f"scbd{i}",
                                     name=f"scbd{i}") for i in range(3)]
            loads = [
                (Wa, dWa), (Wu, dWu), (Aff, dAff), (Wct, dWct), (Wcb, dWcb),
                (Wp1, dWp1), (Wp2, dWp2), (Ib, dIb), (bu, dbu), (bc, dbc),
            ]
            if with_ba:
                loads.append((baW, dbaW))
            for tile_, src in loads:
                nc.sync.dma_start(out=tile_[:], in_=src)
            nc.gpsimd.memset(ones[:], 1.0)
            nc.gpsimd.memset(onesbd[:], 0.0)
            nc.gpsimd.memset(onesbd[0:64, :, 0], 1.0)
            nc.gpsimd.memset(onesbd[64:128, :, 1], 1.0)
            for st in sm_tiles + scbd_tiles:
                nc.gpsimd.memset(st[:], 0.0)

            def load(t):
                xT = wpool.tile([128, 1024], BF, tag="xT")
                atn = wpool.tile([128, 8, 128], BF, tag="atn")
                nc.sync.dma_start(out=xT[:], in_=dET[t:t + 1])
                nc.sync.dma_start(out=atn[:], in_=dAT[t:t + 1])
                return xT, atn

            def phase_a(t, xT, atn):
                """Message passing for all 16 graphs -> (e_T [128,1024], e_n)."""
                # ax node-major directly: (x @ Wa)^T^T per 128-token block
                ps_axn = ppool.tile([128, 8, 128], F32, tag="big")
                for b_ in range(8):
                    nc.tensor.matmul(ps_axn[:, b_, :],
                                     xT[:, b_ * 128:(b_ + 1) * 128], Wa[:],
                                     start=True, stop=not with_ba)
                    if with_ba:
                        nc.tensor.matmul(ps_axn[:, b_, :], ones[:], baW[:],
                                         start=False, stop=True)
                axn = wpool.tile([128, 8, 128], BF, tag="axn")
                nc.scalar.activation(axn[:], ps_axn[:], AF.Relu)

                # ux feature-major (bias per-partition here)
                ps_ux = ppool.tile([128, 2, 512], F32, tag="big")
                nc.tensor.matmul(ps_ux[:, 0, :], Wu[:], xT[:, 0:512])
                nc.tensor.matmul(ps_ux[:, 1, :], Wu[:], xT[:, 512:1024])
                uxT = wpool.tile([128, 1024], BF, tag="uxT")
                nc.scalar.activation(
                    uxT[:].rearrange("p (h c) -> p h c", h=2), ps_ux[:],
                    AF.Relu, bias=bu[:, 0:1])

                # e = An @ ax (feature-major); ux added in finish_a later
                ps_e = ppool.tile([128, 8, 128], F32, tag="big")
                for b_ in range(8):
                    nc.tensor.matmul(ps_e[:, b_, :], axn[:, b_, :], atn[:, b_, :])
                return ps_e, uxT

            def finish_a(t, ps_e, uxT):
                """e_T = ps_e + ux^T; node-major copy; t = (e1 @ Aff)^T.
                Emitted after B2a so the PE transposes never wait on the
                DVE add."""
                e_T = wpool.tile([128, 1024], BF, tag="eT")
                nc.vector.tensor_tensor(
                    out=e_T[:].rearrange("p (b c) -> p b c", b=8), in0=ps_e[:],
                    in1=uxT[:].rearrange("p (b c) -> p b c", b=8), op=ALU.add)
                ps_en = spool.tile([128, 8, 128], BF, tag="s")
                for b_ in range(8):
                    nc.tensor.transpose(ps_en[:, b_, :],
                                        e_T[:, b_ * 128:(b_ + 1) * 128], Ib[:])
                e_n = wpool.tile([128, 8, 128], BF, tag="en")
                nc.vector.tensor_copy(e_n[:], ps_en[:])
                ps_t = spool.tile([128, 512], F32, tag="s")
                nc.tensor.matmul(ps_t[:], Aff[:], e_T[:, 0:512])
                tT = wpool.tile([128, 512], BF, tag="tT")
                nc.scalar.copy(tT[:], ps_t[:])
                return e_T, e_n, tT

            def dump_cols(src_T, t):
                """Debug: write col n=0 of each pair (16 cols) to dgT[t]."""
                gT = wpool.tile([128, 16], F32, tag="gT")
                nc.vector.tensor_copy(
                    gT[:], src_T[:].rearrange("p (c n) -> p c n", n=64)[:, :, 0])
                nc.sync.dma_start(out=dgT[t:t + 1], in_=gT[:])

            def pair_b1(t, e_T, e_n, tT):
                """Affinity scores + softmax (both directions, batched)."""
                sm = sm_tiles[t % 3]
                ps_s = spool.tile([128, 8, 64], F32, tag="s")
                for p in range(G):
                    gg, par = p // 2, p % 2
                    sl = slice(par * 64, (par + 1) * 64)
                    tb = tT[:, p * 64:(p + 1) * 64]
                    eb = e_T[:, 512 + p * 64:512 + (p + 1) * 64]
                    nc.tensor.matmul(ps_s[sl, gg, :], tb, eb,
                                     tile_position=(0, par * 64))
                    nc.tensor.matmul(ps_s[sl, 4 + gg, :], eb, tb,
                                     tile_position=(0, par * 64))

                mx = wpool.tile([128, 8], F32, tag="mx")
                nc.vector.reduce_max(mx[:], ps_s[:], axis=AX.X)
                sb = wpool.tile([128, 8, 64], BF, tag="sb")
                nc.vector.tensor_tensor(
                    out=sb[:], in0=ps_s[:],
                    in1=mx[:].to_broadcast([128, 8, 64]), op=ALU.subtract)
                E = wpool.tile([128, 8, 64], BF, tag="E")
                nc.scalar.activation(E[:], sb[:], AF.Exp)
                den = wpool.tile([128, 8], F32, tag="den")
                nc.vector.reduce_sum(den[:], E[:], axis=AX.X)
                rs = wpool.tile([128, 8], F32, tag="rs")
                nc.vector.reciprocal(rs[:], den[:])
                nc.vector.tensor_tensor(
                    out=sm[0:64, :, 0:64], in0=E[0:64, :, :],
                    in1=rs[0:64, :].to_broadcast([64, 8, 64]), op=ALU.mult)
                nc.vector.tensor_tensor(
                    out=sm[64:128, :, 64:128], in0=E[64:128, :, :],
                    in1=rs[64:128, :].to_broadcast([64, 8, 64]), op=ALU.mult)
                return sm

            def pair_b2a(t, e_T, e_n, sm, ps_smT):
                """Softmax transpose + cross-graph aggregation z."""
                for b_ in range(8):
                    nc.tensor.transpose(ps_smT[:, b_, :], sm[:, b_, :], Ib[:])
                smT = wpool.tile([128, 8, 128], BF, tag="smT")
                nc.vector.tensor_copy(smT[:], ps_smT[:, 0:8, :])

                ps_z = ppool.tile([128, 8, 128], F32, tag="big")
                for gg in range(4):
                    nc.tensor.matmul(ps_z[:, gg, :], e_n[:, 4 + gg, :],
                                     smT[:, gg, :])
                    nc.tensor.matmul(ps_z[:, 4 + gg, :], e_n[:, gg, :],
                                     smT[:, 4 + gg, :])
                zT = wpool.tile([128, 1024], BF, tag="zT")
                nc.scalar.copy(
                    zT[:].rearrange("p (b c) -> p b c", b=8), ps_z[:])
                return zT

            def pair_b2b(t, e_T, e_n, zT):
                """Combine, pooling, output (single stage)."""
                if STAGE == 4:
                    dump_cols(zT, t)
                    return
                ps_n = ppool.tile([128, 2, 512], F32, tag="big")
                for h in range(2):
                    nc.tensor.matmul(ps_n[:, h, :], Wct[:],
                                     e_T[:, h * 512:(h + 1) * 512],
                                     start=True, stop=False)
                    nc.tensor.matmul(ps_n[:, h, :], Wcb[:],
                                     zT[:, h * 512:(h + 1) * 512],
                                     start=False, stop=True)
                nT = wpool.tile([128, 1024], BF, tag="nT")
                nc.scalar.activation(
                    nT[:].rearrange("p (h c) -> p h c", h=2), ps_n[:],
                    AF.Identity, bias=bc[:, 0:1])
                if STAGE == 5:
                    dump_cols(nT, t)
                    return
                ps_nn = spool.tile([128, 8, 128], BF, tag="s")
                for b_ in range(8):
                    nc.tensor.transpose(ps_nn[:, b_, :],
                                        nT[:, b_ * 128:(b_ + 1) * 128], Ib[:])
                n_n = wpool.tile([128, 8, 128], BF, tag="nn")
                nc.vector.tensor_copy(n_n[:], ps_nn[:])
                return nT, n_n

            def pool_head(t, nT, n_n):
                ps_pool = spool.tile([128, 512], F32, tag="s")
                msum = wpool.tile([128, 16], BF, tag="msum")
                with nc.allow_low_precision(reason="bf16 node-mean for ctx"):
                    nc.vector.reduce_sum(
                        msum[:], nT[:].rearrange("p (c n) -> p c n", n=64),
                        axis=AX.X)
                nc.tensor.matmul(ps_pool[:, 0:8], Wp1[:], msum[:, 0:8])
                nc.tensor.matmul(ps_pool[:, 8:16], Wp2[:], msum[:, 8:16])
                ctxT = wpool.tile([128, 16], BF, tag="ctxT")
                nc.scalar.activation(ctxT[:], ps_pool[:, 0:16], AF.Tanh,
                                     scale=1.0 / N)
                return ps_pool, ctxT

            def pool_tail1(t, nT, ps_pool, ctxT):
                """Attention scores -> sigmoid -> block-diag score columns."""
                for c in range(16):
                    par = c % 2
                    nc.tensor.matmul(
                        ps_pool[par * 64:(par + 1) * 64, 16 + c // 2:17 + c // 2],
                        nT[:, c * 64:(c + 1) * 64], ctxT[:, c:c + 1],
                        tile_position=(0, par * 64))
                esc = wpool.tile([128, 8], F32, tag="esc")
                nc.scalar.activation(esc[:], ps_pool[:, 16:24], AF.Exp,
                                     scale=-1.0)
                nc.gpsimd.tensor_scalar_add(esc[:], esc[:], 1.0)
                rsc = wpool.tile([128, 8], F32, tag="rsc")
                nc.vector.reciprocal(rsc[:], esc[:])
                scbd = scbd_tiles[t % 3]
                nc.gpsimd.tensor_copy(scbd[0:64, :, 0], rsc[0:64, :])
                nc.gpsimd.tensor_copy(scbd[64:128, :, 1], rsc[64:128, :])
                return scbd

            def pool_tail2(t, n_n, scbd, ps_pool):
                """Weighted sums one tile later: scbd is a full tile old."""
                for b_ in range(8):
                    nc.tensor.matmul(ps_pool[:, 24 + 2 * b_:26 + 2 * b_],
                                     n_n[:, b_, :], scbd[:, b_, :])
                gT = wpool.tile([128, 16], BF, tag="gTo")
                nc.scalar.copy(gT[:], ps_pool[:, 24:40])
                nc.sync.dma_start(out=dgT[t:t + 1], in_=gT[:])

            # software pipeline: phase A split around B2a; softmax one tile
            # ahead of its consumer; pooling spread over the iteration tail.
            tiles = {}
            tiles[0] = load(0)
            if NT > 1:
                tiles[1] = load(1)
            cur = finish_a(0, *phase_a(0, *tiles.pop(0)))
            cur_sm = None if STAGE == 2 else pair_b1(0, *cur)
            pool_in = {}
            heads = {}
            sc_out = {}
            for t in range(NT + 2):
                if t + 2 < NT:
                    tiles[t + 2] = load(t + 2)
                if t < NT and STAGE != 2:
                    ps_smT = ppool.tile([128, 16, 128], BF, tag="big",
                                        name="ps_smT")
                pa_nxt = (phase_a(t + 1, *tiles.pop(t + 1))
                          if t + 1 < NT else None)
                if t < NT:
                    if STAGE == 2:
                        dump_cols(cur[0], t)
                    else:
                        zT = pair_b2a(t, cur[0], cur[1], cur_sm, ps_smT)
                nxt = finish_a(t + 1, *pa_nxt) if pa_nxt is not None else None
                if 0 <= t - 1 < NT and (t - 1) in pool_in:
                    heads[t - 1] = pool_head(t - 1, *pool_in[t - 1])
                nxt_sm = (pair_b1(t + 1, *nxt)
                          if nxt is not None and STAGE != 2 else None)
                if t < NT and STAGE != 2:
                    r = pair_b2b(t, cur[0], cur[1], zT)
                    if r is not None:
                        pool_in[t] = r
                cur, cur_sm = nxt, nxt_sm
                cur_pool = None
                u = t - 1
                if 0 <= u < NT and u in pool_in:
                    cur_pool = spool.tile([128, 512], F32, tag="s",
                                          name="cur_pool")
                    sc_out[u] = pool_tail1(u, pool_in[u][0], cur_pool,
                                           heads.pop(u)[1])
                u = t - 2
                if 0 <= u < NT and u in sc_out:
                    if cur_pool is None:
                        cur_pool = spool.tile([128, 512], F32, tag="s",
                                              name="cur_pool")
                    pool_tail2(u, pool_in.pop(u)[1], sc_out.pop(u), cur_pool)
    nc.finalize()
    return nc


_BUILT = {}


def _get_nc(n_pairs, with_ba=False):
    key = (n_pairs, with_ba)
    if key not in _BUILT:
        nc = bacc.Bacc("TRN2", target_bir_lowering=False, debug=False,
                       num_devices=NCORES)
        _BUILT[key] = _emit(nc, n_pairs, with_ba)
    return _BUILT[key]


def _prep_side(ml, eT, atn, side, A, emb):
    """Host: bf16 feature-major emb + column-normalized block-diag A^T."""
    bf = ml.bfloat16
    NTt = eT.shape[0]
    ee = np.asarray(emb, np.float32).reshape(NTt, G, 64, 128)
    eT[:, :, side * 512:(side + 1) * 512] = (
        ee.transpose(0, 3, 1, 2).reshape(NTt, 128, 512).astype(bf))
    A = np.asarray(A, np.float32)
    An = A / np.clip(A.sum(axis=1, keepdims=True), 1e-12, None)
    AT = An.transpose(0, 2, 1).reshape(NTt, 4, 2, 64, 64).astype(bf)
    atn[:, 0:64, side * 4:(side + 1) * 4, 0:64] = AT[:, :, 0].transpose(0, 2, 1, 3)
    atn[:, 64:128, side * 4:(side + 1) * 4, 64:128] = AT[:, :, 1].transpose(0, 2, 1, 3)


def kernel(A_src, emb_src, mask_src, A_dst, emb_dst, mask_dst,
           Wa, ba, Wu, bu, Aff, Wc, bc, Wp1, Wp2):
    import ml_dtypes as ml
    bf = ml.bfloat16

    Bt = np.asarray(A_src).shape[0]
    n_pairs = Bt // NCORES
    NTt = Bt // G
    ba = np.asarray(ba, np.float32)
    with_ba = bool(np.abs(ba).max() > 0)
    nc = _get_nc(n_pairs, with_ba)

    eT = np.empty((NTt, 128, 1024), dtype=bf)
    atn = np.zeros((NTt, 128, 8, 128), dtype=bf)
    _prep_side(ml, eT, atn, 0, A_src, emb_src)
    _prep_side(ml, eT, atn, 1, A_dst, emb_dst)

    shared = {
        "Wa": np.asarray(Wa, bf),
        "Wu": np.asarray(Wu, bf),
        "Aff": np.asarray(Aff, bf),
        "Wct": np.ascontiguousarray(np.asarray(Wc, np.float32)[:D]).astype(bf),
        "Wcb": np.ascontiguousarray(np.asarray(Wc, np.float32)[D:]).astype(bf),
        "Wp1": np.asarray(Wp1, bf),
        "Wp2": np.asarray(Wp2, bf),
        "baW": np.tile((ba / 128.0)[None, :], (128, 1)).astype(bf),
        "bu_col": np.ascontiguousarray(np.asarray(bu, np.float32)[:, None]),
        "bc_col": np.ascontiguousarray(np.asarray(bc, np.float32)[:, None]),
        "ident_bf": np.eye(128, dtype=bf),
    }
    NTc = n_pairs // G
    in_maps = []
    for c in range(NCORES):
        sl = slice(c * NTc, (c + 1) * NTc)
        in_maps.append({
            "eT_all": np.ascontiguousarray(eT[sl]),
            "atn_all": np.ascontiguousarray(atn[sl]),
            **shared,
        })
    res = run_bass_kernel_spmd(nc, in_maps, list(range(NCORES)))
    gs = [np.asarray(res.results[c]["gT_all"]).astype(np.float32)
          for c in range(NCORES)]
    gT_all = np.concatenate(gs, axis=0)  # [NTt, 128, 16]
    g1 = gT_all[:, :, 0:8].transpose(0, 2, 1).reshape(Bt, 128)
    g2 = gT_all[:, :, 8:16].transpose(0, 2, 1).reshape(Bt, 128)
    return (np.ascontiguousarray(g1), np.ascontiguousarray(g2))
